# revision 38
# baseline (speedup 1.0000x reference)
"""DeepSeek-style MoE (E=8, top-6, silu-GLU experts + shared expert) on 8
TRN2 NeuronCores.

Sharding (hardcoded, matches spec sharding_hint):
  - tokens (B*S = 4096) split in 2 halves across core groups {0-3}, {4-7}
  - experts split 4-ways within each group: core handles expert pair
    {2p, 2p+1}, p = core % 4, plus a 256-wide slice of the shared expert's
    FS=1024 dimension.
  - router replicated (with per-core column permutation so that "my"
    experts are always gate columns 0 and 1 -> identical SPMD program).
  - host-side unshard: sum the 4 partial [D, tok] outputs per token half,
    transpose, concat.

Precision plan (abs-max rel err budget 2e-2; measured ~1.5e-2):
  - router: 3-term bf16 decomposition  z = xb@rwh + xl@rwh + xb@rwl
    (xl/rwl are bf16 residuals; dropped xl@rwl term is ~2^-18 of z, so
    top-6 selection is exact -- a tie flip would cost ~3e-2).
  - expert gate/up: e4m3 fp8 with DoubleRow perf mode (2x PE rate).
    x is pre-scaled by 16, W by 64; the 1/1024 is folded into the silu
    activation scale (gate path) and the gate broadcast tile (up path).
  - shared expert + all down-projections + h: bf16.
"""

import numpy as np
from contextlib import ExitStack

# ---- model dims (hardcoded from the problem spec) ----
B, S, D, E, F, FS = 2, 2048, 1024, 8, 512, 1024
TOP_K = 6
T = B * S                     # 4096 tokens total
TOK = T // 2                  # 2048 tokens per core (token half)
FL = 2 * F + FS // 4          # 1280 local F: expert0 | expert1 | shared slice
DC = D // 128                 # 8 contraction chunks
FC = FL // 128                # 10 local F chunks (0-3: e0, 4-7: e1, 8-9: shared)
NTH = 4                       # token sub-blocks per core
TTH = TOK // NTH              # 1024 tokens per sub-block
TT = TTH // 128               # 8 token tiles per sub-block
TS = TTH // 512               # 2 512-token slices per sub-block

S_X8 = 16.0                   # fp8 scale for x
S_W8 = 64.0                   # fp8 scale for expert gate/up weights
INV_GU = 1.0 / (S_X8 * S_W8)  # removed via silu-scale / bcast tile
N_WARM = 12                   # PE warmup matmuls (ride out DMA queue init)

_CACHE = {}


def _build_nc():
    import concourse.bacc as bacc
    import concourse.tile as tile
    from concourse import mybir, masks

    f32 = mybir.dt.float32
    f32r = mybir.dt.float32r
    bf16 = mybir.dt.bfloat16
    f8 = mybir.dt.float8e4
    AF = mybir.ActivationFunctionType
    OP = mybir.AluOpType
    AX = mybir.AxisListType
    DR = mybir.MatmulPerfMode.DoubleRow

    nc = bacc.Bacc("TRN2", target_bir_lowering=False, debug=False)

    # chunk-major x: [th, p, c*TTH + t] = x-ish[c*128+p, th*TTH+t]
    xq8 = nc.dram_tensor("xq8", [NTH, 128, DC * TTH], f8,
                         kind="ExternalInput").ap()   # e4m3(16*x)
    xqb = nc.dram_tensor("xqb", [NTH, 128, DC * TTH], bf16,
                         kind="ExternalInput").ap()   # bf16(x)
    xql = nc.dram_tensor("xql", [NTH, 128, DC * TTH], bf16,
                         kind="ExternalInput").ap()   # bf16(x - bf16(x))
    # router weights, bf16 hi/lo, device layout: rw2[p, c*16 + w*8 + e]
    rw2 = nc.dram_tensor("rw2", [128, DC * 2 * E], bf16,
                         kind="ExternalInput").ap()
    # block identity: idq[32j+i, i] = 1 — summing combiner for col groups
    idq = nc.dram_tensor("idq", [128, E], f32, kind="ExternalInput").ap()
    # expert gate/up (chunks 0-7): wge8[fc, p, c*128+f'] = 64*Wg[c*128+p, fc*128+f']
    wge8 = nc.dram_tensor("wge8", [8, 128, D], f8, kind="ExternalInput").ap()
    wue8 = nc.dram_tensor("wue8", [8, 128, D], f8, kind="ExternalInput").ap()
    # shared gate/up (chunks 8-9), bf16, same per-chunk layout
    wgs = nc.dram_tensor("wgs", [2, 128, D], bf16, kind="ExternalInput").ap()
    wus = nc.dram_tensor("wus", [2, 128, D], bf16, kind="ExternalInput").ap()
    # wd[dc, p, fc*128+d'] = Wd[fc*128+p, dc*128+d']  (bf16)
    wd = nc.dram_tensor("wd", [DC, 128, FL], bf16, kind="ExternalInput").ap()
    yT = nc.dram_tensor("yT", [D, TOK], f32, kind="ExternalOutput").ap()

    rw2_v = rw2.rearrange("p (c w e) -> p c w e", c=DC, w=2)  # [128,DC,2,E]

    with tile.TileContext(nc) as tc:
        with ExitStack() as ctx:
            ep = ctx.enter_context

            cpool = ep(tc.tile_pool(name="consts", bufs=1))
            x8p = ep(tc.tile_pool(name="x8", bufs=2))
            xbp = ep(tc.tile_pool(name="xb", bufs=2))
            xlp = ep(tc.tile_pool(name="xl", bufs=2))
            hp = ep(tc.tile_pool(name="hT", bufs=1))
            gp = ep(tc.tile_pool(name="gate", bufs=2))
            gtp = ep(tc.tile_pool(name="gateT", bufs=4))
            bcp = ep(tc.tile_pool(name="bcast", bufs=4))
            sp = ep(tc.tile_pool(name="glu", bufs=3))
            yp = ep(tc.tile_pool(name="yout", bufs=3))

            # 8 PSUM banks: router/bcast misc(1) + g(3) + u(2) + y(2)
            mps = ep(tc.tile_pool(name="mps", bufs=1, space="PSUM"))
            gps_p = ep(tc.tile_pool(name="gps", bufs=3, space="PSUM"))
            ups_p = ep(tc.tile_pool(name="ups", bufs=2, space="PSUM"))
            yps = ep(tc.tile_pool(name="yps", bufs=2, space="PSUM"))

            warm = cpool.tile([128, 512], bf16)
            nc.vector.memset(warm[:], 0.5)
            identity = cpool.tile([128, 128], f32)
            masks.make_identity(nc, identity[:])
            identity_r = cpool.tile([128, 128], f32r)
            nc.vector.tensor_copy(identity_r[:], identity[:])
            ones_f = cpool.tile([1, 128], f32)
            nc.vector.memset(ones_f[:], INV_GU)
            ones = cpool.tile([1, 128], f32r)
            nc.vector.tensor_copy(ones[:], ones_f[:])
            rw_sb = cpool.tile([128, DC, 2, E], bf16)
            nc.sync.dma_start(rw_sb[:], rw2_v)
            idq_sb = cpool.tile([128, E], f32)
            nc.sync.dma_start(idq_sb[:], idq)

            # resident weights (scalar-engine DMA queue, loaded once)
            wgs_sb = cpool.tile([128, 2, D], bf16)
            nc.scalar.dma_start(wgs_sb[:], wgs.rearrange("s p d -> p s d"))
            wus_sb = cpool.tile([128, 2, D], bf16)
            nc.scalar.dma_start(wus_sb[:], wus.rearrange("s p d -> p s d"))
            wge8_sb = cpool.tile([128, 8, D], f8)
            nc.gpsimd.dma_start(wge8_sb[:], wge8.rearrange("f p d -> p f d"))
            wue8_sb = cpool.tile([128, 8, D], f8)
            nc.gpsimd.dma_start(wue8_sb[:], wue8.rearrange("f p d -> p f d"))
            wd_sb = cpool.tile([128, DC, FL], bf16)
            nc.gpsimd.dma_start(wd_sb[:], wd.rearrange("d p f -> p d f"))

            # ---- pre-trigger ACT function tables (silu, exp) so the lazy
            # ACT_TABLE_LOADs don't land in front of the first real silu ----
            tdum = cpool.tile([1, 128], bf16)
            nc.scalar.activation(tdum[:], warm[0:1, 0:128], AF.Silu)
            nc.scalar.activation(tdum[:], warm[0:1, 0:128], AF.Exp)
            nc.scalar.copy(tdum[:], warm[0:1, 0:128])

            # ---- PE warmup: dep-free matmuls on a memset tile. Rides out
            # the ~9us DMA queue init and climbs the p-state ladder. ----
            for i in range(N_WARM):
                w_ps = yps.tile([128, 512], f32, tag="y", name=f"warm{i}")
                nc.tensor.matmul(
                    w_ps[:], warm[:, 0:128], warm[:], start=True, stop=True)

            for th in range(NTH):
                t0 = th * TTH  # token offset of this sub-block within core

                # ---- stage x (sync queue): xb halves first (shared GLU),
                # then xl (router), then x8 (expert fp8) ----
                xb = xbp.tile([128, DC, TTH], bf16, tag="xb")
                for g in range(2):
                    nc.sync.dma_start(
                        xb[:, g * 4:(g + 1) * 4, :],
                        xqb[th, :, g * 4 * TTH:(g + 1) * 4 * TTH]
                        .rearrange("p (c t) -> p c t", c=4))
                xl = xlp.tile([128, DC, TTH], bf16, tag="xl")
                nc.sync.dma_start(
                    xl[:], xql[th].rearrange("p (c t) -> p c t", c=DC))
                x8 = x8p.tile([128, DC, TTH], f8, tag="x8")
                nc.sync.dma_start(
                    x8[:], xq8[th].rearrange("p (c t) -> p c t", c=DC))
                if th == 0:
                    for h in range(2):
                        fsl = slice(h * 4, (h + 1) * 4)
                        nc.sync.dma_start(
                            wge8_sb[:, fsl, :],
                            wge8[fsl].rearrange("f p d -> p f d"))
                        nc.sync.dma_start(
                            wue8_sb[:, fsl, :],
                            wue8[fsl].rearrange("f p d -> p f d"))
                    nc.sync.dma_start(wd_sb[:],
                                      wd.rearrange("d p f -> p d f"))
                    nc.sync.dma_start(wd8_sb[:],
                                      wd8.rearrange("d p f -> p d f"))

                hT = hp.tile([128, FC, TTH], bf16, tag="hT")

                def glu_shared(fc):
                    sfc = fc - 8
                    for ts in range(TS):
                        sl = slice(ts * 512, (ts + 1) * 512)
                        g_ps = gps_p.tile([128, 512], f32, tag="g",
                                          name=f"g{th}_{fc}_{ts}")
                        for c in range(DC):
                            nc.tensor.matmul(
                                g_ps[:],
                                wgs_sb[:, sfc, c * 128:(c + 1) * 128],
                                xb[:, c, sl],
                                start=(c == 0), stop=(c == DC - 1),
                            )
                        u_ps = ups_p.tile([128, 512], f32, tag="u",
                                          name=f"u{th}_{fc}_{ts}")
                        for c in range(DC):
                            nc.tensor.matmul(
                                u_ps[:],
                                wus_sb[:, sfc, c * 128:(c + 1) * 128],
                                xb[:, c, sl],
                                start=(c == 0), stop=(c == DC - 1),
                            )
                        sg = sp.tile([128, 512], bf16, tag="sg",
                                     name=f"sg{th}_{fc}_{ts}")
                        nc.scalar.activation(sg[:], g_ps[:], AF.Silu)
                        nc.vector.tensor_mul(hT[:, fc, sl], sg[:], u_ps[:])

                # ---- shared chunk 8 (needs only xb); router; shared 9 ----
                glu_shared(8)

                # router: transposed scores zT[e-groups, tok], 3-term bf16.
                # col group j holds chunks {2j, 2j+1} at partitions 32j..32j+16:
                # one 16-wide pass computes xb@[rwh|rwl] (rows i<8: rwh-part,
                # rows 8..16: rwl-part), one 8-wide pass adds xl@rwh. idq sums
                # rows {i, i+8} of each group into e = i mod 8.
                zT_sb = gp.tile([128, TTH], f32, tag="zT_sb")
                for ts in range(TS):
                    sl = slice(ts * 512, (ts + 1) * 512)
                    zt = mps.tile([128, 512], f32, tag="m",
                                  name=f"zt{th}_{ts}")
                    for j in range(4):
                        c0, c1 = 2 * j, 2 * j + 1
                        nc.tensor.matmul(
                            zt[32 * j:32 * j + 16, :],
                            rw_sb[:, c0, :, :].rearrange("p w e -> p (w e)"),
                            xb[:, c0, sl],
                            start=True, stop=False,
                            tile_position=(0, 32 * j),
                        )
                        for c in (c0, c1):
                            nc.tensor.matmul(
                                zt[32 * j:32 * j + 8, :],
                                rw_sb[:, c, 0, :],
                                xl[:, c, sl],
                                start=False, stop=False,
                                tile_position=(0, 32 * j),
                            )
                        nc.tensor.matmul(
                            zt[32 * j:32 * j + 16, :],
                            rw_sb[:, c1, :, :].rearrange("p w e -> p (w e)"),
                            xb[:, c1, sl],
                            start=False, stop=True,
                            tile_position=(0, 32 * j),
                        )
                    nc.vector.tensor_copy(zT_sb[:, sl], zt[:])
                    if ts == 0:
                        glu_shared(9)

                # combine + transpose the 4 col-group partials back to
                # token-major z[tok, e]: z[:, t, :] = zT_slice.T @ idq
                z_ps = mps.tile([128, TT, E], f32, tag="m", name=f"z{th}")
                for t in range(TT):
                    nc.tensor.matmul(
                        z_ps[:, t, :],
                        zT_sb[:, t * 128:(t + 1) * 128],
                        idq_sb[:],
                        start=True, stop=True,
                    )

                # ---- softmax (no max-sub; |z| is small) + top-6 mask ----
                z_sb = gp.tile([128, TT, E], f32, tag="z_sb")
                nc.vector.tensor_copy(z_sb[:], z_ps[:])
                e_sb = gp.tile([128, TT, E], f32, tag="e_sb")
                nc.scalar.activation(e_sb[:], z_sb[:], AF.Exp)
                esum = gp.tile([128, TT], f32, tag="esum")
                nc.vector.tensor_reduce(esum[:], e_sb[:], AX.X, OP.add)
                rcp = gp.tile([128, TT], f32, tag="rcp")
                nc.vector.reciprocal(rcp[:], esum[:])
                m1 = gp.tile([128, TT], f32, tag="m1")
                nc.vector.tensor_reduce(m1[:], z_sb[:], AX.X, OP.min)
                eq = gp.tile([128, TT, E], f32, tag="eq")
                nc.vector.tensor_tensor(
                    eq[:], z_sb[:], m1[:].broadcast_to([128, TT, E]),
                    op=OP.is_equal)
                zb = gp.tile([128, TT, E], f32, tag="zb")
                nc.vector.scalar_tensor_tensor(
                    zb[:], eq[:], 1e30, z_sb[:], op0=OP.mult, op1=OP.add
                )
                m2 = gp.tile([128, TT], f32, tag="m2")
                nc.vector.tensor_reduce(m2[:], zb[:], AX.X, OP.min)
                gate = gp.tile([128, TT, E], f32r, tag="gate")
                # keep = z > m2 ? 1 : 0 ; gate = keep * (e * rcp)
                nc.vector.tensor_tensor(
                    gate[:], z_sb[:], m2[:].broadcast_to([128, TT, E]),
                    op=OP.is_gt)
                nc.vector.tensor_tensor(
                    e_sb[:], e_sb[:], rcp[:].broadcast_to([128, TT, E]),
                    op=OP.mult)
                nc.vector.tensor_mul(gate[:], gate[:], e_sb[:])

                # ---- gate columns 0,1 -> broadcast [128, tok] tiles.
                # bcast = gate * INV_GU (ones tile holds INV_GU), which
                # removes the fp8 input scaling from the up path. ----
                bcast = {}
                for e in range(2):
                    for ts in range(TS):
                        gt_ps = mps.tile([1, 512], f32r, tag="m",
                                         name=f"gt{th}_{e}_{ts}")
                        for tq in range(4):
                            t = ts * 4 + tq
                            nc.tensor.transpose(
                                gt_ps[0:1, tq * 128:(tq + 1) * 128],
                                gate[:, t, e:e + 1],
                                identity_r[:],
                            )
                        gt_sb = gtp.tile([1, 512], f32r, tag="gt_sb",
                                         name=f"gtsb{th}_{e}_{ts}")
                        nc.vector.tensor_copy(gt_sb[:], gt_ps[:])
                        bc_ps = mps.tile([128, 512], f32, tag="m",
                                         name=f"bc{th}_{e}_{ts}")
                        nc.tensor.matmul(
                            bc_ps[:], ones[0:1, :], gt_sb[0:1, :], start=True, stop=True
                        )
                        bc_sb = bcp.tile([128, 512], f32, tag="bc_sb",
                                         name=f"bcsb{th}_{e}_{ts}")
                        nc.scalar.copy(bc_sb[:], bc_ps[:])
                        bcast[(e, ts)] = bc_sb

                # ---- expert GLU, e4m3 DoubleRow (2x PE rate) ----
                for fc in range(8):
                    for ts in range(TS):
                        sl = slice(ts * 512, (ts + 1) * 512)
                        g_ps = gps_p.tile([128, 512], f32, tag="g",
                                          name=f"g{th}_{fc}_{ts}")
                        for c2 in range(4):
                            nc.tensor.matmul(
                                g_ps[:],
                                wge8_sb[:, fc, c2 * 256:(c2 + 1) * 256]
                                .rearrange("p (two f) -> p two f", two=2),
                                x8[:, 2 * c2:2 * c2 + 2, sl],
                                perf_mode=DR,
                                start=(c2 == 0), stop=(c2 == 3),
                            )
                        u_ps = ups_p.tile([128, 512], f32, tag="u",
                                          name=f"u{th}_{fc}_{ts}")
                        for c2 in range(4):
                            nc.tensor.matmul(
                                u_ps[:],
                                wue8_sb[:, fc, c2 * 256:(c2 + 1) * 256]
                                .rearrange("p (two f) -> p two f", two=2),
                                x8[:, 2 * c2:2 * c2 + 2, sl],
                                perf_mode=DR,
                                start=(c2 == 0), stop=(c2 == 3),
                            )
                        sg = sp.tile([128, 512], bf16, tag="sg",
                                     name=f"sg{th}_{fc}_{ts}")
                        nc.scalar.activation(sg[:], g_ps[:], AF.Silu,
                                             scale=INV_GU)
                        ug = sp.tile([128, 512], bf16, tag="ug",
                                     name=f"ug{th}_{fc}_{ts}")
                        nc.vector.tensor_mul(
                            ug[:], bcast[(fc // 4, ts)][:], u_ps[:]
                        )
                        nc.vector.tensor_mul(hT[:, fc, sl], sg[:], ug[:])

                # ---- down-proj: yT[d, tok] (bf16 weights, resident) ----
                for dc in range(DC):
                    for ts in range(TS):
                        sl = slice(ts * 512, (ts + 1) * 512)
                        y_ps = yps.tile([128, 512], f32, tag="y")
                        for fc in range(FC):
                            nc.tensor.matmul(
                                y_ps[:],
                                wd_sb[:, dc, fc * 128:(fc + 1) * 128],
                                hT[:, fc, sl],
                                start=(fc == 0), stop=(fc == FC - 1),
                            )
                        y_sb = yp.tile([128, 512], f32, tag="y_sb")
                        nc.scalar.copy(y_sb[:], y_ps[:])
                        nc.gpsimd.dma_start(
                            yT[dc * 128:(dc + 1) * 128,
                               t0 + ts * 512:t0 + (ts + 1) * 512],
                            y_sb[:],
                        )

    nc.compile()
    return nc


def _get_nc():
    if "nc" not in _CACHE:
        _CACHE["nc"] = _build_nc()
    return _CACHE["nc"]


def _shard_inputs(hidden_states, router_w, w_gate, w_up, w_down,
                  ws_gate, ws_up, ws_down):
    import ml_dtypes
    f8np = ml_dtypes.float8_e4m3
    bfnp = ml_dtypes.bfloat16

    x = np.asarray(hidden_states, np.float32).reshape(T, D)
    idq = np.zeros((128, E), np.float32)
    for j in range(4):
        for i in range(2 * E):
            idq[32 * j + i, i % E] = 1.0
    in_maps = []
    for c in range(8):
        th, p = divmod(c, 4)
        e0, e1 = 2 * p, 2 * p + 1
        perm = [e0, e1] + [e for e in range(E) if e not in (e0, e1)]
        fs = slice(p * (FS // 4), (p + 1) * (FS // 4))
        xT_c = x[th * TOK:(th + 1) * TOK, :].T          # [D, TOK]
        # chunk-major: [NTH, 128, DC*TTH]
        x_cm = (xT_c.reshape(DC, 128, NTH, TTH).transpose(2, 1, 0, 3)
                .reshape(NTH, 128, DC * TTH))
        xq8_c = np.ascontiguousarray((x_cm * S_X8).astype(f8np))
        xqb_c = np.ascontiguousarray(x_cm.astype(bfnp))
        xql_c = np.ascontiguousarray(
            (x_cm - xqb_c.astype(np.float32)).astype(bfnp))
        rw_c = np.asarray(router_w, np.float32)[:, perm]
        rwh = rw_c.astype(bfnp)
        rwl = (rw_c - rwh.astype(np.float32)).astype(bfnp)
        # device layout: rw2[p, c*16 + w*8 + e] = rw[w][c*128+p, e]
        rw2_c = np.ascontiguousarray(
            np.stack([rwh, rwl], axis=0)            # [2, D, E]
            .reshape(2, DC, 128, E).transpose(2, 1, 0, 3)
            .reshape(128, DC * 2 * E))
        wg_exp = np.concatenate([w_gate[e0], w_gate[e1]], axis=1,
                                dtype=np.float32)          # [D, 1024]
        wu_exp = np.concatenate([w_up[e0], w_up[e1]], axis=1,
                                dtype=np.float32)
        # wge8[fc, p, c*128+f'] = 64*Wg[c*128+p, fc*128+f']  (fc 0..7)
        wge8_c = np.ascontiguousarray(
            (wg_exp * S_W8).reshape(DC, 128, 8, 128).transpose(2, 1, 0, 3)
            .reshape(8, 128, D).astype(f8np))
        wue8_c = np.ascontiguousarray(
            (wu_exp * S_W8).reshape(DC, 128, 8, 128).transpose(2, 1, 0, 3)
            .reshape(8, 128, D).astype(f8np))
        wgs_c = np.ascontiguousarray(
            np.asarray(ws_gate[:, fs], np.float32)
            .reshape(DC, 128, 2, 128).transpose(2, 1, 0, 3)
            .reshape(2, 128, D).astype(bfnp))
        wus_c = np.ascontiguousarray(
            np.asarray(ws_up[:, fs], np.float32)
            .reshape(DC, 128, 2, 128).transpose(2, 1, 0, 3)
            .reshape(2, 128, D).astype(bfnp))
        wd_full = np.concatenate(
            [w_down[e0], w_down[e1], ws_down[fs, :]], axis=0, dtype=np.float32)
        # wd[dc, p, fc*128+d'] = wd_full[fc*128+p, dc*128+d']
        wd_c = np.ascontiguousarray(
            wd_full.reshape(FC, 128, DC, 128).transpose(2, 1, 0, 3)
            .reshape(DC, 128, FL).astype(bfnp))
        in_maps.append({"xq8": xq8_c, "xqb": xqb_c, "xql": xql_c,
                        "rw2": rw2_c, "idq": idq,
                        "wge8": wge8_c, "wue8": wue8_c, "wgs": wgs_c,
                        "wus": wus_c, "wd": wd_c})
    return in_maps


def _run(in_maps, **kwargs):
    from concourse import bass_utils
    nc = _get_nc()
    return bass_utils.run_bass_kernel_spmd(
        nc, in_maps, core_ids=list(range(8)), **kwargs
    )


def _unshard(results):
    parts = [r["yT"] for r in results]
    y0 = parts[0] + parts[1] + parts[2] + parts[3]   # [D, TOK]
    y1 = parts[4] + parts[5] + parts[6] + parts[7]
    y = np.concatenate([y0.T, y1.T], axis=0)         # [T, D]
    return np.ascontiguousarray(y.reshape(B, S, D).astype(np.float32))


def kernel(**inputs):
    in_maps = _shard_inputs(**inputs)
    res = _run(in_maps)
    return _unshard(res.results)


def kernel_profiled(**inputs):
    """Like kernel(), but with NTFF tracing; returns (y, BassKernelResults)."""
    in_maps = _shard_inputs(**inputs)
    res = _run(in_maps, trace=True)
    return _unshard(res.results), res
